# revision 21
# baseline (speedup 1.0000x reference)
"""Trainium2 Bass kernel for the MixExpertAttentionQNetwork problem.

Strategy
--------
Data-parallel over the batch: 4096 rows are split as 512 rows per NeuronCore
across 8 cores; all weights are replicated.  No collectives.

Device-side math (per core, batch chunk BC=512, activations held in
[feature, batch] layout, matmuls in bf16 with fp32 PSUM accumulation):

  rep  = (relu(x @ W0 + b0)) @ W1 + b1                      x = concat(s, act)
  per expert e:
    hk_e = relu(rep @ Wk0_e + bk0_e)
    logits[:, e] = sum_t onehot[b,t] * (hk_e @ F_e)[b,t] + FB[task_b, e]
       where F_e = Wk1_e @ tanh(emb).T  (folded on host, [DH, NT]) —
       this removes the entire keys=hk@Wk1 matmul (~17% of FLOPs), exact
       by associativity since query = tanh(emb[task]) has only NT rows.
  attn = softmax(logits); loss terms = clip(log(attn+1e-10), -6, 0)
  per expert e:
    hv_e  = relu(rep @ Wv0_e + bv0_e)
    vals_e = hv_e @ Wv1_e + bv1_e            (bias via rank-1 ones matmul)
    acc  += attn[:, e] * vals_e              (fused DVE scalar_tensor_tensor)
  q = relu(relu(acc @ Wt0 + bt0) @ Wt1 + bt1) @ Wt2 + bt2

Host only prepares layouts (transpose/pad/cast) and the small O(weights)
fold F_e / FB; all O(batch) compute runs on device.
"""

import sys

import numpy as np
import ml_dtypes

sys.path.insert(0, "/opt/trn_rl_repo")

import concourse.bass as bass  # noqa: E402
import concourse.mybir as mybir  # noqa: E402
import concourse.tile as tile  # noqa: E402
from concourse import bacc  # noqa: E402
from concourse.bass_utils import run_bass_kernel_spmd  # noqa: E402

bf16 = ml_dtypes.bfloat16
F32 = mybir.dt.float32
F32R = mybir.dt.float32r
BF = mybir.dt.bfloat16
AF = mybir.ActivationFunctionType
ALU = mybir.AluOpType
AX = mybir.AxisListType

B, OBS, ACTD, NT, E = 4096, 512, 64, 10, 16
DREP, DH, DK = 1024, 1024, 512
NCORES = 8
BC = B // NCORES          # 512 rows per core
BT = BC // 128            # 4 batch tiles per core
DIN = OBS + ACTD          # 576
KT_IN = 5                 # padded input feature tiles (640)
DINP = KT_IN * 128
MT = DREP // 128          # 8 rep feature tiles
HT = DH // 128            # 8 hidden tiles

_CACHE = {}


def _emit(nc, tc, io):
    """Emit the per-core program (SPMD: identical on all cores)."""
    import os
    from contextlib import ExitStack

    STAGE = int(os.environ.get("KSTAGE", "7"))
    KVAR = os.environ.get("KVAR", "")

    with ExitStack() as ctx:
        const = ctx.enter_context(tc.tile_pool(name="const", bufs=1))
        wpool = ctx.enter_context(tc.tile_pool(name="wpool", bufs=2))
        repw = ctx.enter_context(tc.tile_pool(name="repw", bufs=1))
        wv1pool = ctx.enter_context(tc.tile_pool(name="wv1pool", bufs=2))
        ev = ctx.enter_context(tc.tile_pool(name="ev", bufs=3))
        accp = ctx.enter_context(tc.tile_pool(name="accp", bufs=2))
        ps = ctx.enter_context(tc.tile_pool(name="ps", bufs=3, space="PSUM"))
        acps = ctx.enter_context(tc.tile_pool(name="acps", bufs=1, space="PSUM"))

        # ---- constants / persistent tiles -------------------------------
        def load(name, shape, dt):
            t = const.tile(shape, dt, tag=name)
            nc.sync.dma_start(out=t[:], in_=io[name][:])
            return t

        xt = load("xt", [128, KT_IN * BC], F32R)
        oh = load("oh", [128, BT * NT], BF)
        oht = load("oht", [NT, BC], BF)
        ft = load("ft", [128, HT * E * NT], BF)
        fb = load("fb", [NT, E], BF)
        bv1s = load("bv1s", [1, E * DK], BF)
        b0t = load("b0t", [128, MT], F32)
        b1t = load("b1t", [128, MT], F32)
        bk0t = load("bk0t", [128, E * HT], F32)
        bv0t = load("bv0t", [128, E * HT], F32)
        bt0t = load("bt0t", [128, 4], F32)
        bt1t = load("bt1t", [128, 2], F32)
        bt2t = load("bt2t", [1, 1], F32)
        wt0 = load("wt0", [128, 4 * 512], F32R)
        wt1 = load("wt1", [128, 4 * 256], F32R)
        wt2 = load("wt2", [128, 2], F32R)
        ident = load("ident", [128, 128], F32)

        ones1 = const.tile([1, 128], BF, tag="ones1")
        nc.vector.memset(ones1[:], 1.0)
        zerob = const.tile([128, 1], F32, tag="zerob")
        nc.vector.memset(zerob[:], 0.0)
        epsb = const.tile([128, 1], F32, tag="epsb")
        nc.vector.memset(epsb[:], 1e-10)

        rep = const.tile([128, MT * BC], BF, tag="rep")
        h0 = const.tile([128, MT * BC], F32R, tag="h0")
        logits = const.tile([128, BT * E], F32, tag="logits")
        flog = const.tile([128, BT * E], F32, tag="flog")
        attn = const.tile([128, BT * E], F32, tag="attn")
        loss = const.tile([128, BT], F32, tag="loss")
        twt = const.tile([128, 4 * BC], F32R, tag="twt")
        h1 = const.tile([128, 4 * BC], F32R, tag="h1")
        h2 = const.tile([128, 2 * BC], F32R, tag="h2")
        qsb = const.tile([1, BC], F32, tag="qsb")

        # ---- rep MLP ----------------------------------------------------
        w0 = repw.tile([128, KT_IN * DREP], F32R, tag="repw")
        nc.sync.dma_start(out=w0[:], in_=io["w0"][:])
        for m in range(MT):
            p = ps.tile([128, BC], F32, tag="mm")
            for k in range(KT_IN):
                nc.tensor.matmul(
                    p[:], (w0[:, k * DREP + m * 128:k * DREP + m * 128 + 128]),
                    (xt[:, k * BC:(k + 1) * BC]),
                    start=(k == 0), stop=(k == KT_IN - 1))
            nc.scalar.activation(h0[:, m * BC:(m + 1) * BC], p[:], AF.Relu,
                                 bias=b0t[:, m:m + 1])
        w1 = repw.tile([128, MT * DREP], F32R, tag="repw")
        nc.sync.dma_start(out=w1[:], in_=io["w1"][:])
        for m in range(MT):
            p = ps.tile([128, BC], F32, tag="mm")
            for k in range(MT):
                nc.tensor.matmul(
                    p[:], (w1[:, k * DREP + m * 128:k * DREP + m * 128 + 128]),
                    (h0[:, k * BC:(k + 1) * BC]),
                    start=(k == 0), stop=(k == MT - 1))
            nc.scalar.activation(rep[:, m * BC:(m + 1) * BC], p[:], AF.Identity,
                                 bias=b1t[:, m:m + 1])

        if STAGE < 2:
            nc.vector.memset(loss[:], 0.0)
            nc.vector.memset(qsb[:], 0.0)
            nc.sync.dma_start(out=io["lossout"][:], in_=loss[:])
            nc.sync.dma_start(out=io["qout"][:], in_=qsb[:])
            return

        # ---- flogit base: FB[task_b, e] via onehot matmul ---------------
        for bt in range(BT):
            fp = acps.tile([128, E], F32, tag=f"ac{bt}")
            nc.tensor.matmul(fp[:], oht[:, bt * 128:(bt + 1) * 128], fb[:],
                             start=True, stop=True)
            nc.scalar.copy(flog[:, bt * E:(bt + 1) * E], fp[:])

        if STAGE < 3:
            nc.vector.memset(loss[:], 0.0)
            nc.vector.memset(qsb[:], 0.0)
            nc.sync.dma_start(out=io["lossout"][:], in_=loss[:])
            nc.sync.dma_start(out=io["qout"][:], in_=qsb[:])
            return

        # ---- pass 1: hk + logits per expert -----------------------------
        for e in range(E):
            wk = wpool.tile([128, HT * DH], BF, tag="bigw")
            nc.sync.dma_start(out=wk[:], in_=io["wk0"][e])
            lg = [acps.tile([128, NT], F32, tag=f"ac{bt}", name=f"lg{bt}")
                  for bt in range(BT)]
            for h in range(HT):
                p = ps.tile([128, BC], F32, tag="mm")
                for k in range(MT):
                    nc.tensor.matmul(
                        p[:], wk[:, k * DH + h * 128:k * DH + h * 128 + 128],
                        rep[:, k * BC:(k + 1) * BC],
                        start=(k == 0), stop=(k == MT - 1))
                hk = ev.tile([128, BC], BF, tag="hk")
                nc.scalar.activation(hk[:], p[:], AF.Relu,
                                     bias=bk0t[:, e * HT + h:e * HT + h + 1])
                if KVAR == "a":
                    continue
                fcol = h * E * NT + e * NT
                for bt in range(BT):
                    nc.tensor.matmul(
                        lg[bt][:], hk[:, bt * 128:(bt + 1) * 128],
                        ft[:, fcol:fcol + NT],
                        start=(h == 0), stop=(h == HT - 1),
                        skip_group_check=True)
            if KVAR == "a":
                continue
            for bt in range(BT):
                if KVAR == "b":
                    nc.scalar.copy(logits[:, bt * E + e:bt * E + e + 1],
                                   lg[bt][:, 0:1])
                    continue
                sel = ev.tile([128, NT], F32, tag="sel")
                nc.vector.tensor_tensor(sel[:], lg[bt][:],
                                        oh[:, bt * NT:(bt + 1) * NT], ALU.mult)
                nc.vector.reduce_sum(logits[:, bt * E + e:bt * E + e + 1],
                                     sel[:], axis=AX.X)
        if KVAR == "a":
            nc.vector.memset(logits[:], 0.0)

        if STAGE < 4:
            nc.vector.memset(loss[:], 0.0)
            nc.vector.memset(qsb[:], 0.0)
            nc.vector.tensor_copy(loss[:, 0:1], logits[:, 0:1])
            nc.sync.dma_start(out=io["lossout"][:], in_=loss[:])
            nc.sync.dma_start(out=io["qout"][:], in_=qsb[:])
            return

        # ---- softmax + loss --------------------------------------------
        for bt in range(BT):
            lt = logits[:, bt * E:(bt + 1) * E]
            nc.vector.tensor_tensor(lt, lt, flog[:, bt * E:(bt + 1) * E],
                                    ALU.add)
            mx = ev.tile([128, 1], F32, tag="mx")
            nc.vector.reduce_max(mx[:], lt, axis=AX.X)
            sh = ev.tile([128, E], F32, tag="sh")
            nc.vector.tensor_scalar(sh[:], lt, mx[:], None, op0=ALU.subtract)
            ex = ev.tile([128, E], F32, tag="ex")
            se = ev.tile([128, 1], F32, tag="se")
            nc.scalar.activation(ex[:], sh[:], AF.Exp, bias=zerob[:, 0:1],
                                 accum_out=se[:])
            rs = ev.tile([128, 1], F32, tag="rs")
            nc.vector.reciprocal(rs[:], se[:])
            at = attn[:, bt * E:(bt + 1) * E]
            nc.vector.tensor_scalar(at, ex[:], rs[:], None, op0=ALU.mult)
            ln = ev.tile([128, E], F32, tag="ln")
            nc.scalar.activation(ln[:], at, AF.Ln, bias=epsb[:, 0:1])
            cl = ev.tile([128, E], F32, tag="cl")
            nc.vector.tensor_scalar(cl[:], ln[:], -6.0, 0.0,
                                    op0=ALU.max, op1=ALU.min)
            nc.vector.reduce_sum(loss[:, bt:bt + 1], cl[:], axis=AX.X)
        nc.sync.dma_start(out=io["lossout"][:], in_=loss[:])

        if STAGE < 5:
            nc.vector.memset(qsb[:], 0.0)
            nc.sync.dma_start(out=io["qout"][:], in_=qsb[:])
            return

        # ---- pass 2: hv + vals + weighted combine -----------------------
        acc_prev = None
        for e in range(E):
            wv = wpool.tile([128, HT * DH], BF, tag="bigw")
            nc.sync.dma_start(out=wv[:], in_=io["wv0"][e])
            wv1 = wv1pool.tile([128, HT * DK], BF, tag="wv1")
            nc.sync.dma_start(out=wv1[:], in_=io["wv1"][e])
            vps = [acps.tile([128, DK], F32, tag=f"ac{bt}", name=f"vps{bt}")
                   for bt in range(BT)]
            for h in range(HT):
                p = ps.tile([128, BC], F32, tag="mm")
                for k in range(MT):
                    nc.tensor.matmul(
                        p[:], wv[:, k * DH + h * 128:k * DH + h * 128 + 128],
                        rep[:, k * BC:(k + 1) * BC],
                        start=(k == 0), stop=(k == MT - 1))
                hv = ev.tile([128, BC], BF, tag="hv")
                nc.scalar.activation(hv[:], p[:], AF.Relu,
                                     bias=bv0t[:, e * HT + h:e * HT + h + 1])
                for bt in range(BT):
                    nc.tensor.matmul(
                        vps[bt][:], hv[:, bt * 128:(bt + 1) * 128],
                        wv1[:, h * DK:(h + 1) * DK],
                        start=(h == 0), stop=False, skip_group_check=True)
            acc = accp.tile([128, BT * DK], F32, tag="acc")
            for bt in range(BT):
                nc.tensor.matmul(vps[bt][:], ones1[:],
                                 bv1s[:, e * DK:(e + 1) * DK],
                                 start=False, stop=True, skip_group_check=True)
                a_sc = attn[:, bt * E + e:bt * E + e + 1]
                dst = acc[:, bt * DK:(bt + 1) * DK]
                if e == 0:
                    nc.vector.tensor_scalar(dst, vps[bt][:], a_sc, None,
                                            op0=ALU.mult)
                else:
                    nc.vector.scalar_tensor_tensor(
                        dst, vps[bt][:], a_sc,
                        acc_prev[:, bt * DK:(bt + 1) * DK],
                        op0=ALU.mult, op1=ALU.add)
            acc_prev = acc

        if STAGE < 6:
            nc.vector.memset(qsb[:], 0.0)
            nc.vector.tensor_copy(qsb[:, 0:1], acc_prev[0:1, 0:1])
            nc.sync.dma_start(out=io["qout"][:], in_=qsb[:])
            return

        # ---- transpose tower input to [k, b] ---------------------------
        for bt in range(BT):
            for kt in range(4):
                tp = ps.tile([128, 128], F32, tag="mm")
                nc.tensor.transpose(
                    tp[:], acc_prev[:, bt * DK + kt * 128:bt * DK + kt * 128 + 128],
                    ident[:])
                nc.scalar.copy(twt[:, kt * BC + bt * 128:kt * BC + bt * 128 + 128],
                               tp[:])

        if STAGE < 7:
            nc.vector.memset(qsb[:], 0.0)
            nc.vector.tensor_copy(qsb[:, 0:1], twt[0:1, 0:1])
            nc.sync.dma_start(out=io["qout"][:], in_=qsb[:])
            return

        # ---- tower ------------------------------------------------------
        for n in range(4):
            p = ps.tile([128, BC], F32, tag="mm")
            for k in range(4):
                nc.tensor.matmul(
                    p[:], (wt0[:, k * 512 + n * 128:k * 512 + n * 128 + 128]),
                    (twt[:, k * BC:(k + 1) * BC]),
                    start=(k == 0), stop=(k == 3))
            nc.scalar.activation(h1[:, n * BC:(n + 1) * BC], p[:], AF.Relu,
                                 bias=bt0t[:, n:n + 1])
        for n in range(2):
            p = ps.tile([128, BC], F32, tag="mm")
            for k in range(4):
                nc.tensor.matmul(
                    p[:], (wt1[:, k * 256 + n * 128:k * 256 + n * 128 + 128]),
                    (h1[:, k * BC:(k + 1) * BC]),
                    start=(k == 0), stop=(k == 3))
            nc.scalar.activation(h2[:, n * BC:(n + 1) * BC], p[:], AF.Relu,
                                 bias=bt1t[:, n:n + 1])
        qp = ps.tile([1, BC], F32, tag="mm")
        for k in range(2):
            nc.tensor.matmul(qp[:], (wt2[:, k:k + 1]),
                             (h2[:, k * BC:(k + 1) * BC]),
                             start=(k == 0), stop=(k == 1))
        nc.scalar.activation(qsb[:], qp[:], AF.Identity, bias=bt2t[:, 0:1])
        nc.sync.dma_start(out=io["qout"][:], in_=qsb[:])


def _build():
    if "nc" in _CACHE:
        return _CACHE["nc"]
    nc = bacc.Bacc("TRN2", target_bir_lowering=False, debug=False)
    io = {}

    def inp(name, shape, dt):
        io[name] = nc.dram_tensor(name, shape, dt, kind="ExternalInput")

    inp("xt", [128, KT_IN * BC], F32R)
    inp("oh", [128, BT * NT], BF)
    inp("oht", [NT, BC], BF)
    inp("w0", [128, KT_IN * DREP], F32R)
    inp("w1", [128, MT * DREP], F32R)
    inp("wk0", [E, 128, HT * DH], BF)
    inp("wv0", [E, 128, HT * DH], BF)
    inp("wv1", [E, 128, HT * DK], BF)
    inp("ft", [128, HT * E * NT], BF)
    inp("fb", [NT, E], BF)
    inp("bv1s", [1, E * DK], BF)
    inp("b0t", [128, MT], F32)
    inp("b1t", [128, MT], F32)
    inp("bk0t", [128, E * HT], F32)
    inp("bv0t", [128, E * HT], F32)
    inp("bt0t", [128, 4], F32)
    inp("bt1t", [128, 2], F32)
    inp("bt2t", [1, 1], F32)
    inp("wt0", [128, 4 * 512], F32R)
    inp("wt1", [128, 4 * 256], F32R)
    inp("wt2", [128, 2], F32R)
    inp("ident", [128, 128], F32)
    io["qout"] = nc.dram_tensor("qout", [1, BC], F32, kind="ExternalOutput")
    io["lossout"] = nc.dram_tensor("lossout", [128, BT], F32,
                                   kind="ExternalOutput")

    with tile.TileContext(nc) as tc:
        _emit(nc, tc, io)
    nc.compile()
    _CACHE["nc"] = nc
    return nc


def _prep(inputs):
    f32 = np.float32

    def g(name):
        return np.asarray(inputs[name], f32)

    def bft(a):
        return np.ascontiguousarray(np.asarray(a, f32).astype(bf16))

    st, act = g("state"), g("act")
    x = np.concatenate([st[:, :OBS], act], axis=1)
    xpad = np.zeros((B, DINP), f32)
    xpad[:, :DIN] = x
    oh_full = st[:, OBS:OBS + NT]

    tanh_emb = np.tanh(g("emb"))                      # [NT, DK]
    F = np.einsum("ehk,tk->eht", g("Wk1"), tanh_emb)  # [E, DH, NT]
    FB = tanh_emb @ g("bk1").T                        # [NT, E]

    W0p = np.zeros((DINP, DREP), f32)
    W0p[:DIN] = g("rep_W0")

    rep_common = {
        "w0": np.ascontiguousarray(
            W0p.reshape(KT_IN, 128, DREP).transpose(1, 0, 2)
            .reshape(128, KT_IN * DREP)),
        "w1": np.ascontiguousarray(
            g("rep_W1").reshape(MT, 128, DREP).transpose(1, 0, 2)
            .reshape(128, MT * DREP)),
        "wk0": bft(g("Wk0").reshape(E, MT, 128, DH).transpose(0, 2, 1, 3)
                   .reshape(E, 128, MT * DH)),
        "wv0": bft(g("Wv0").reshape(E, MT, 128, DH).transpose(0, 2, 1, 3)
                   .reshape(E, 128, MT * DH)),
        "wv1": bft(g("Wv1").reshape(E, HT, 128, DK).transpose(0, 2, 1, 3)
                   .reshape(E, 128, HT * DK)),
        "ft": bft(F.reshape(E, HT, 128, NT).transpose(2, 1, 0, 3)
                  .reshape(128, HT * E * NT)),
        "fb": bft(FB),
        "bv1s": bft(g("bv1").reshape(1, E * DK)),
        "b0t": np.ascontiguousarray(g("rep_b0").reshape(MT, 128).T),
        "b1t": np.ascontiguousarray(g("rep_b1").reshape(MT, 128).T),
        "bk0t": np.ascontiguousarray(
            g("bk0").reshape(E, HT, 128).transpose(2, 0, 1).reshape(128, E * HT)),
        "bv0t": np.ascontiguousarray(
            g("bv0").reshape(E, HT, 128).transpose(2, 0, 1).reshape(128, E * HT)),
        "bt0t": np.ascontiguousarray(g("bt0").reshape(4, 128).T),
        "bt1t": np.ascontiguousarray(g("bt1").reshape(2, 128).T),
        "bt2t": np.ascontiguousarray(g("bt2").reshape(1, 1)),
        "wt0": np.ascontiguousarray(
            g("Wt0").reshape(4, 128, 512).transpose(1, 0, 2)
            .reshape(128, 4 * 512)),
        "wt1": np.ascontiguousarray(
            g("Wt1").reshape(4, 128, 256).transpose(1, 0, 2)
            .reshape(128, 4 * 256)),
        "wt2": np.ascontiguousarray(
            g("Wt2").reshape(2, 128, 1).transpose(1, 0, 2)
            .reshape(128, 2)),
        "ident": np.eye(128, dtype=f32),
    }

    in_maps = []
    for c in range(NCORES):
        rows = slice(c * BC, (c + 1) * BC)
        xc = xpad[rows]                                # [BC, DINP]
        ohc = oh_full[rows]                            # [BC, NT]
        m = dict(rep_common)
        m["xt"] = np.ascontiguousarray(
            xc.T.reshape(KT_IN, 128, BC).transpose(1, 0, 2)
            .reshape(128, KT_IN * BC))
        m["oh"] = bft(ohc.reshape(BT, 128, NT).transpose(1, 0, 2)
                      .reshape(128, BT * NT))
        m["oht"] = bft(ohc.T)
        in_maps.append(m)
    return in_maps


def _run(inputs, trace=False):
    nc = _build()
    in_maps = _prep(inputs)
    res = run_bass_kernel_spmd(nc, in_maps, list(range(NCORES)), trace=trace)
    q = np.concatenate([res.results[c]["qout"][0] for c in range(NCORES)])
    tot = sum(float(res.results[c]["lossout"].sum()) for c in range(NCORES))
    loss = np.float32(-0.3 * tot / B)
    return (q.astype(np.float32), loss), res


def kernel(**inputs):
    (q, loss), _ = _run(inputs, trace=False)
    return q, loss


# revision 29
# speedup vs baseline: 1.2801x; 1.2801x over previous
"""Trainium2 Bass kernel for the MixExpertAttentionQNetwork problem.

Strategy
--------
Data-parallel over the batch: 4096 rows are split as 512 rows per NeuronCore
across 8 cores; all weights are replicated.  No collectives.

Device-side math (per core, batch chunk BC=512, activations held in
[feature, batch] layout, matmuls in bf16 with fp32 PSUM accumulation):

  rep  = (relu(x @ W0 + b0)) @ W1 + b1                      x = concat(s, act)
  per expert e:
    hk_e = relu(rep @ Wk0_e + bk0_e)
    logits[:, e] = sum_t onehot[b,t] * (hk_e @ F_e)[b,t] + FB[task_b, e]
       where F_e = Wk1_e @ tanh(emb).T  (folded on host, [DH, NT]) —
       this removes the entire keys=hk@Wk1 matmul (~17% of FLOPs), exact
       by associativity since query = tanh(emb[task]) has only NT rows.
  attn = softmax(logits); loss terms = clip(log(attn+1e-10), -6, 0)
  per expert e:
    hv_e  = relu(rep @ Wv0_e + bv0_e)
    vals_e = hv_e @ Wv1_e + bv1_e            (bias via rank-1 ones matmul)
    acc  += attn[:, e] * vals_e              (fused DVE scalar_tensor_tensor)
  q = relu(relu(acc @ Wt0 + bt0) @ Wt1 + bt1) @ Wt2 + bt2

Host only prepares layouts (transpose/pad/cast) and the small O(weights)
fold F_e / FB; all O(batch) compute runs on device.
"""

import sys

import numpy as np
import ml_dtypes

sys.path.insert(0, "/opt/trn_rl_repo")

import concourse.bass as bass  # noqa: E402
import concourse.mybir as mybir  # noqa: E402
import concourse.tile as tile  # noqa: E402
from concourse import bacc  # noqa: E402
from concourse.bass_utils import run_bass_kernel_spmd  # noqa: E402

bf16 = ml_dtypes.bfloat16
F32 = mybir.dt.float32
F32R = mybir.dt.float32r
BF = mybir.dt.bfloat16
AF = mybir.ActivationFunctionType
ALU = mybir.AluOpType
AX = mybir.AxisListType

B, OBS, ACTD, NT, E = 4096, 512, 64, 10, 16
DREP, DH, DK = 1024, 1024, 512
NCORES = 8
BC = B // NCORES          # 512 rows per core
BT = BC // 128            # 4 batch tiles per core
DIN = OBS + ACTD          # 576
KT_IN = 5                 # padded input feature tiles (640)
DINP = KT_IN * 128
MT = DREP // 128          # 8 rep feature tiles
HT = DH // 128            # 8 hidden tiles

_CACHE = {}


def _emit(nc, tc, io):
    """Emit the per-core program (SPMD: identical on all cores)."""
    import os
    from contextlib import ExitStack

    STAGE = int(os.environ.get("KSTAGE", "7"))

    with ExitStack() as ctx:
        const = ctx.enter_context(tc.tile_pool(name="const", bufs=1))
        wpool = ctx.enter_context(tc.tile_pool(name="wpool", bufs=3))
        repw = ctx.enter_context(tc.tile_pool(name="repw", bufs=4))
        wv1pool = ctx.enter_context(tc.tile_pool(name="wv1pool", bufs=2))
        ev = ctx.enter_context(tc.tile_pool(name="ev", bufs=3))
        accp = ctx.enter_context(tc.tile_pool(name="accp", bufs=2))
        ps = ctx.enter_context(tc.tile_pool(name="ps", bufs=3, space="PSUM"))
        acps = ctx.enter_context(tc.tile_pool(name="acps", bufs=1, space="PSUM"))

        # ---- constants / persistent tiles -------------------------------
        def load(name, shape, dt):
            t = const.tile(shape, dt, tag=name)
            nc.sync.dma_start(out=t[:], in_=io[name][:])
            return t

        w0m = [repw.tile([128, KT_IN * 128], F32R, tag="w0m", name=f"w0m{m}")
               for m in range(MT)]
        nc.sync.dma_start(out=w0m[0][:], in_=io["w0"][:, 0:KT_IN * 128])
        xt = const.tile([128, KT_IN * BC], F32R, tag="xt")
        for k in range(KT_IN):
            nc.sync.dma_start(out=xt[:, k * BC:(k + 1) * BC],
                              in_=io["xt"][:, k * BC:(k + 1) * BC])
        b0t = load("b0t", [128, MT], F32)
        b1t = load("b1t", [128, MT], F32)

        zerob = const.tile([128, 1], F32, tag="zerob")
        nc.vector.memset(zerob[:], 0.0)
        epsb = const.tile([128, 1], F32, tag="epsb")
        nc.vector.memset(epsb[:], 1e-10)

        rep = const.tile([128, MT * BC], BF, tag="rep")
        h0 = const.tile([128, MT * BC], F32R, tag="h0")
        logits = const.tile([128, BT * E], F32, tag="logits")
        flog = const.tile([128, BT * E], F32, tag="flog")
        attn = const.tile([128, BT * E], F32, tag="attn")
        loss = const.tile([128, BT], F32, tag="loss")
        twt = const.tile([128, 4 * BC], F32R, tag="twt")
        h1 = const.tile([128, 4 * BC], F32R, tag="h1")
        h2 = const.tile([128, 2 * BC], F32R, tag="h2")
        qsb = const.tile([1, BC], F32, tag="qsb")

        # ---- rep MLP (per-m weight chunks, m-major dram layout) ---------
        for m in range(1, MT):
            nc.sync.dma_start(out=w0m[m][:],
                              in_=io["w0"][:, m * KT_IN * 128:(m + 1) * KT_IN * 128])
        w1m = [repw.tile([128, MT * 128], F32R, tag="w1m", name=f"w1m{m}")
               for m in range(MT)]
        for m in range(MT):
            nc.sync.dma_start(out=w1m[m][:],
                              in_=io["w1"][:, m * MT * 128:(m + 1) * MT * 128])
        wk_pre = [wpool.tile([128, HT * DH], BF, tag="bigw", name=f"wkpre{e}")
                  for e in range(2)]
        for e in range(2):
            for h in range(HT):
                nc.sync.dma_start(out=wk_pre[e][:, h * DH:(h + 1) * DH],
                                  in_=io["wk0"][e, :, h * DH:(h + 1) * DH])
        for m in range(MT):
            p = ps.tile([128, BC], F32, tag="mm")
            for k in range(KT_IN):
                nc.tensor.matmul(
                    p[:], (w0m[m][:, k * 128:(k + 1) * 128]),
                    (xt[:, k * BC:(k + 1) * BC]),
                    start=(k == 0), stop=(k == KT_IN - 1))
            nc.scalar.activation(h0[:, m * BC:(m + 1) * BC], p[:], AF.Relu,
                                 bias=b0t[:, m:m + 1])
        for m in range(MT):
            p = ps.tile([128, BC], F32, tag="mm")
            for k in range(MT):
                nc.tensor.matmul(
                    p[:], (w1m[m][:, k * 128:(k + 1) * 128]),
                    (h0[:, k * BC:(k + 1) * BC]),
                    start=(k == 0), stop=(k == MT - 1))
            nc.scalar.activation(rep[:, m * BC:(m + 1) * BC], p[:], AF.Identity,
                                 bias=b1t[:, m:m + 1])

        # deferred constants (not needed by the rep phase)
        ft = load("ft", [128, HT * E * NT], BF)
        oh = load("oh", [128, BT * NT], BF)
        bk0t = load("bk0t", [128, E * HT], F32)
        ident = load("ident", [128, 128], F32)
        oht = load("oht", [NT, BC], BF)
        fb = load("fb", [NT, E], BF)
        bv0t = load("bv0t", [128, E * HT], F32)
        bv1m = load("bv1m", [E, DK], BF)
        bt0t = load("bt0t", [128, 4], F32)
        bt1t = load("bt1t", [128, 2], F32)
        bt2t = load("bt2t", [1, 1], F32)
        wt0 = load("wt0", [128, 4 * 512], F32R)
        wt1 = load("wt1", [128, 4 * 256], F32R)
        wt2 = load("wt2", [128, 2], F32R)

        if STAGE < 2:
            nc.vector.memset(loss[:], 0.0)
            nc.vector.memset(qsb[:], 0.0)
            nc.sync.dma_start(out=io["lossout"][:], in_=loss[:])
            nc.sync.dma_start(out=io["qout"][:], in_=qsb[:])
            return

        # ---- flogit base: FB[task_b, e] via onehot matmul ---------------
        for bt in range(BT):
            fp = acps.tile([128, E], F32, tag=f"ac{bt}")
            nc.tensor.matmul(fp[:], oht[:, bt * 128:(bt + 1) * 128], fb[:],
                             start=True, stop=True)
            nc.scalar.copy(flog[:, bt * E:(bt + 1) * E], fp[:])

        if STAGE < 3:
            nc.vector.memset(loss[:], 0.0)
            nc.vector.memset(qsb[:], 0.0)
            nc.sync.dma_start(out=io["lossout"][:], in_=loss[:])
            nc.sync.dma_start(out=io["qout"][:], in_=qsb[:])
            return

        # ---- pass 1: hk + logits per expert -----------------------------
        # P_e[t, b] = sum_h F_e[h, t] * hk[h, b]  (FT stationary: LDW is 10
        # cols, nearly free; hk streams at N=512).  Then transpose per
        # b-chunk and select with the onehot rows.
        for e in range(E):
            if e < 2:
                wk = wk_pre[e]
            else:
                wk = wpool.tile([128, HT * DH], BF, tag="bigw")
                for h in range(HT):
                    nc.sync.dma_start(out=wk[:, h * DH:(h + 1) * DH],
                                      in_=io["wk0"][e, :, h * DH:(h + 1) * DH])
            pe_ps = acps.tile([NT, BC], F32, tag="pe")
            for h in range(HT):
                p = ps.tile([128, BC], F32, tag="mm")
                for k in range(MT):
                    nc.tensor.matmul(
                        p[:], wk[:, h * DH + k * 128:h * DH + k * 128 + 128],
                        rep[:, k * BC:(k + 1) * BC],
                        start=(k == 0), stop=(k == MT - 1))
                hk = ev.tile([128, BC], BF, tag="hk")
                nc.scalar.activation(hk[:], p[:], AF.Relu,
                                     bias=bk0t[:, e * HT + h:e * HT + h + 1])
                fcol = h * E * NT + e * NT
                nc.tensor.matmul(pe_ps[:], ft[:, fcol:fcol + NT], hk[:],
                                 start=(h == 0), stop=(h == HT - 1),
                                 skip_group_check=True)
            pe_sb = accp.tile([NT, BC], F32, tag="pesb")
            nc.scalar.copy(pe_sb[:], pe_ps[:])
            for bt in range(BT):
                tp = acps.tile([128, NT], F32, tag="pe", name="tp")
                nc.tensor.transpose(tp[:], pe_sb[:, bt * 128:(bt + 1) * 128],
                                    ident[0:NT, 0:NT])
                sel = ev.tile([128, NT], F32, tag="sel")
                nc.vector.tensor_tensor(sel[:], tp[:],
                                        oh[:, bt * NT:(bt + 1) * NT], ALU.mult)
                nc.vector.reduce_sum(logits[:, bt * E + e:bt * E + e + 1],
                                     sel[:], axis=AX.X)

        if STAGE < 4:
            nc.vector.memset(loss[:], 0.0)
            nc.vector.memset(qsb[:], 0.0)
            nc.vector.tensor_copy(loss[:, 0:1], logits[:, 0:1])
            nc.sync.dma_start(out=io["lossout"][:], in_=loss[:])
            nc.sync.dma_start(out=io["qout"][:], in_=qsb[:])
            return

        # ---- softmax + loss --------------------------------------------
        for bt in range(BT):
            lt = logits[:, bt * E:(bt + 1) * E]
            nc.vector.tensor_tensor(lt, lt, flog[:, bt * E:(bt + 1) * E],
                                    ALU.add)
            mx = ev.tile([128, 1], F32, tag="mx")
            nc.vector.reduce_max(mx[:], lt, axis=AX.X)
            sh = ev.tile([128, E], F32, tag="sh")
            nc.vector.tensor_scalar(sh[:], lt, mx[:], None, op0=ALU.subtract)
            ex = ev.tile([128, E], F32, tag="ex")
            se = ev.tile([128, 1], F32, tag="se")
            nc.scalar.activation(ex[:], sh[:], AF.Exp, bias=zerob[:, 0:1],
                                 accum_out=se[:])
            rs = ev.tile([128, 1], F32, tag="rs")
            nc.vector.reciprocal(rs[:], se[:])
            at = attn[:, bt * E:(bt + 1) * E]
            nc.vector.tensor_scalar(at, ex[:], rs[:], None, op0=ALU.mult)
            ln = ev.tile([128, E], F32, tag="ln")
            nc.scalar.activation(ln[:], at, AF.Ln, bias=epsb[:, 0:1])
            cl = ev.tile([128, E], F32, tag="cl")
            nc.vector.tensor_scalar(cl[:], ln[:], -6.0, 0.0,
                                    op0=ALU.max, op1=ALU.min)
            nc.vector.reduce_sum(loss[:, bt:bt + 1], cl[:], axis=AX.X)
        nc.sync.dma_start(out=io["lossout"][:], in_=loss[:])

        # bias term of the combine: sum_e attn[b,e] * bv1[e,k] = attnT.T@bv1
        attnt = const.tile([E, BC], BF, tag="attnt")
        bias_sb = const.tile([128, BT * DK], F32, tag="bias_sb")
        for bt in range(BT):
            tpa = acps.tile([E, 128], F32, tag="ac0", name="tpa")
            nc.tensor.transpose(tpa[:], attn[:, bt * E:(bt + 1) * E], ident[:])
            nc.scalar.copy(attnt[:, bt * 128:(bt + 1) * 128], tpa[:])
        for bt in range(BT):
            bp = acps.tile([128, DK], F32, tag="ac1", name="bp")
            nc.tensor.matmul(bp[:], attnt[:, bt * 128:(bt + 1) * 128],
                             bv1m[:], start=True, stop=True)
            nc.scalar.copy(bias_sb[:, bt * DK:(bt + 1) * DK], bp[:])

        if STAGE < 5:
            nc.vector.memset(qsb[:], 0.0)
            nc.sync.dma_start(out=io["qout"][:], in_=qsb[:])
            return

        # ---- pass 2: hv + vals + weighted combine -----------------------
        acc_prev = None
        for e in range(E):
            wv = wpool.tile([128, HT * DH], BF, tag="bigw")
            wv1 = wv1pool.tile([128, HT * DK], BF, tag="wv1")
            for h in range(HT):
                nc.sync.dma_start(out=wv[:, h * DH:(h + 1) * DH],
                                  in_=io["wv0"][e, :, h * DH:(h + 1) * DH])
                nc.sync.dma_start(out=wv1[:, h * DK:(h + 1) * DK],
                                  in_=io["wv1"][e, :, h * DK:(h + 1) * DK])
            vps = [acps.tile([128, DK], F32, tag=f"ac{bt}", name=f"vps{bt}")
                   for bt in range(BT)]
            for h in range(HT):
                p = ps.tile([128, BC], F32, tag="mm")
                for k in range(MT):
                    nc.tensor.matmul(
                        p[:], wv[:, h * DH + k * 128:h * DH + k * 128 + 128],
                        rep[:, k * BC:(k + 1) * BC],
                        start=(k == 0), stop=(k == MT - 1))
                hv = ev.tile([128, BC], BF, tag="hv")
                nc.scalar.activation(hv[:], p[:], AF.Relu,
                                     bias=bv0t[:, e * HT + h:e * HT + h + 1])
                for bt in range(BT):
                    nc.tensor.matmul(
                        vps[bt][:], hv[:, bt * 128:(bt + 1) * 128],
                        wv1[:, h * DK:(h + 1) * DK],
                        start=(h == 0), stop=(h == HT - 1),
                        skip_group_check=True)
            acc = accp.tile([128, BT * DK], F32, tag="acc")
            for bt in range(BT):
                a_sc = attn[:, bt * E + e:bt * E + e + 1]
                dst = acc[:, bt * DK:(bt + 1) * DK]
                prev = (bias_sb if e == 0 else acc_prev)
                nc.vector.scalar_tensor_tensor(
                    dst, vps[bt][:], a_sc,
                    prev[:, bt * DK:(bt + 1) * DK],
                    op0=ALU.mult, op1=ALU.add)
            acc_prev = acc

        if STAGE < 6:
            nc.vector.memset(qsb[:], 0.0)
            nc.vector.tensor_copy(qsb[:, 0:1], acc_prev[0:1, 0:1])
            nc.sync.dma_start(out=io["qout"][:], in_=qsb[:])
            return

        # ---- transpose tower input to [k, b] ---------------------------
        for bt in range(BT):
            for kt in range(4):
                tp = ps.tile([128, 128], F32, tag="mm")
                nc.tensor.transpose(
                    tp[:], acc_prev[:, bt * DK + kt * 128:bt * DK + kt * 128 + 128],
                    ident[:])
                nc.scalar.copy(twt[:, kt * BC + bt * 128:kt * BC + bt * 128 + 128],
                               tp[:])

        if STAGE < 7:
            nc.vector.memset(qsb[:], 0.0)
            nc.vector.tensor_copy(qsb[:, 0:1], twt[0:1, 0:1])
            nc.sync.dma_start(out=io["qout"][:], in_=qsb[:])
            return

        # ---- tower ------------------------------------------------------
        for n in range(4):
            p = ps.tile([128, BC], F32, tag="mm")
            for k in range(4):
                nc.tensor.matmul(
                    p[:], (wt0[:, k * 512 + n * 128:k * 512 + n * 128 + 128]),
                    (twt[:, k * BC:(k + 1) * BC]),
                    start=(k == 0), stop=(k == 3))
            nc.scalar.activation(h1[:, n * BC:(n + 1) * BC], p[:], AF.Relu,
                                 bias=bt0t[:, n:n + 1])
        for n in range(2):
            p = ps.tile([128, BC], F32, tag="mm")
            for k in range(4):
                nc.tensor.matmul(
                    p[:], (wt1[:, k * 256 + n * 128:k * 256 + n * 128 + 128]),
                    (h1[:, k * BC:(k + 1) * BC]),
                    start=(k == 0), stop=(k == 3))
            nc.scalar.activation(h2[:, n * BC:(n + 1) * BC], p[:], AF.Relu,
                                 bias=bt1t[:, n:n + 1])
        qp = ps.tile([1, BC], F32, tag="mm")
        for k in range(2):
            nc.tensor.matmul(qp[:], (wt2[:, k:k + 1]),
                             (h2[:, k * BC:(k + 1) * BC]),
                             start=(k == 0), stop=(k == 1))
        nc.scalar.activation(qsb[:], qp[:], AF.Identity, bias=bt2t[:, 0:1])
        nc.sync.dma_start(out=io["qout"][:], in_=qsb[:])


def _declare_io(nc):
    io = {}

    def inp(name, shape, dt):
        io[name] = nc.dram_tensor(name, shape, dt, kind="ExternalInput")

    inp("xt", [128, KT_IN * BC], F32R)
    inp("oh", [128, BT * NT], BF)
    inp("oht", [NT, BC], BF)
    inp("w0", [128, KT_IN * DREP], F32R)
    inp("w1", [128, MT * DREP], F32R)
    inp("wk0", [E, 128, HT * DH], BF)
    inp("wv0", [E, 128, HT * DH], BF)
    inp("wv1", [E, 128, HT * DK], BF)
    inp("ft", [128, HT * E * NT], BF)
    inp("fb", [NT, E], BF)
    inp("bv1m", [E, DK], BF)
    inp("b0t", [128, MT], F32)
    inp("b1t", [128, MT], F32)
    inp("bk0t", [128, E * HT], F32)
    inp("bv0t", [128, E * HT], F32)
    inp("bt0t", [128, 4], F32)
    inp("bt1t", [128, 2], F32)
    inp("bt2t", [1, 1], F32)
    inp("wt0", [128, 4 * 512], F32R)
    inp("wt1", [128, 4 * 256], F32R)
    inp("wt2", [128, 2], F32R)
    inp("ident", [128, 128], F32)
    io["qout"] = nc.dram_tensor("qout", [1, BC], F32, kind="ExternalOutput")
    io["lossout"] = nc.dram_tensor("lossout", [128, BT], F32,
                                   kind="ExternalOutput")
    return io


def _build():
    if "nc" in _CACHE:
        return _CACHE["nc"]
    nc = bacc.Bacc("TRN2", target_bir_lowering=False, debug=False)
    io = _declare_io(nc)
    with tile.TileContext(nc) as tc:
        _emit(nc, tc, io)
    nc.compile()
    _CACHE["nc"] = nc
    return nc


def _prep(inputs):
    f32 = np.float32

    def g(name):
        return np.asarray(inputs[name], f32)

    def bft(a):
        return np.ascontiguousarray(np.asarray(a, f32).astype(bf16))

    st, act = g("state"), g("act")
    x = np.concatenate([st[:, :OBS], act], axis=1)
    xpad = np.zeros((B, DINP), f32)
    xpad[:, :DIN] = x
    oh_full = st[:, OBS:OBS + NT]

    tanh_emb = np.tanh(g("emb"))                      # [NT, DK]
    F = np.einsum("ehk,tk->eht", g("Wk1"), tanh_emb)  # [E, DH, NT]
    FB = tanh_emb @ g("bk1").T                        # [NT, E]

    W0p = np.zeros((DINP, DREP), f32)
    W0p[:DIN] = g("rep_W0")

    rep_common = {
        "w0": np.ascontiguousarray(
            W0p.reshape(KT_IN, 128, MT, 128).transpose(1, 2, 0, 3)
            .reshape(128, KT_IN * DREP)),
        "w1": np.ascontiguousarray(
            g("rep_W1").reshape(MT, 128, MT, 128).transpose(1, 2, 0, 3)
            .reshape(128, MT * DREP)),
        "wk0": bft(g("Wk0").reshape(E, MT, 128, HT, 128)
                   .transpose(0, 2, 3, 1, 4).reshape(E, 128, MT * DH)),
        "wv0": bft(g("Wv0").reshape(E, MT, 128, HT, 128)
                   .transpose(0, 2, 3, 1, 4).reshape(E, 128, MT * DH)),
        "wv1": bft(g("Wv1").reshape(E, HT, 128, DK).transpose(0, 2, 1, 3)
                   .reshape(E, 128, HT * DK)),
        "ft": bft(F.reshape(E, HT, 128, NT).transpose(2, 1, 0, 3)
                  .reshape(128, HT * E * NT)),
        "fb": bft(FB),
        "bv1m": bft(g("bv1")),
        "b0t": np.ascontiguousarray(g("rep_b0").reshape(MT, 128).T),
        "b1t": np.ascontiguousarray(g("rep_b1").reshape(MT, 128).T),
        "bk0t": np.ascontiguousarray(
            g("bk0").reshape(E, HT, 128).transpose(2, 0, 1).reshape(128, E * HT)),
        "bv0t": np.ascontiguousarray(
            g("bv0").reshape(E, HT, 128).transpose(2, 0, 1).reshape(128, E * HT)),
        "bt0t": np.ascontiguousarray(g("bt0").reshape(4, 128).T),
        "bt1t": np.ascontiguousarray(g("bt1").reshape(2, 128).T),
        "bt2t": np.ascontiguousarray(g("bt2").reshape(1, 1)),
        "wt0": np.ascontiguousarray(
            g("Wt0").reshape(4, 128, 512).transpose(1, 0, 2)
            .reshape(128, 4 * 512)),
        "wt1": np.ascontiguousarray(
            g("Wt1").reshape(4, 128, 256).transpose(1, 0, 2)
            .reshape(128, 4 * 256)),
        "wt2": np.ascontiguousarray(
            g("Wt2").reshape(2, 128, 1).transpose(1, 0, 2)
            .reshape(128, 2)),
        "ident": np.eye(128, dtype=f32),
    }

    in_maps = []
    for c in range(NCORES):
        rows = slice(c * BC, (c + 1) * BC)
        xc = xpad[rows]                                # [BC, DINP]
        ohc = oh_full[rows]                            # [BC, NT]
        m = dict(rep_common)
        m["xt"] = np.ascontiguousarray(
            xc.T.reshape(KT_IN, 128, BC).transpose(1, 0, 2)
            .reshape(128, KT_IN * BC))
        m["oh"] = bft(ohc.reshape(BT, 128, NT).transpose(1, 0, 2)
                      .reshape(128, BT * NT))
        m["oht"] = bft(ohc.T)
        in_maps.append(m)
    return in_maps


def _run(inputs, trace=False):
    nc = _build()
    in_maps = _prep(inputs)
    res = run_bass_kernel_spmd(nc, in_maps, list(range(NCORES)), trace=trace)
    q = np.concatenate([res.results[c]["qout"][0] for c in range(NCORES)])
    tot = sum(float(res.results[c]["lossout"].sum()) for c in range(NCORES))
    loss = np.float32(-0.3 * tot / B)
    return (q.astype(np.float32), loss), res


def kernel(**inputs):
    (q, loss), _ = _run(inputs, trace=False)
    return q, loss
